# revision 48
# baseline (speedup 1.0000x reference)
"""LoRA-MoE fused kernel for 8x Trainium2 NeuronCores (Bass/Tile).

Math (per batch sample b, data-parallel across 8 cores):
    g_b    = gate_w @ mean_s(x_b) + gate_b                      # [E]
    out_b  = x_b @ W^T + ((x_b @ A^T) * g_rep) @ Bt + bias      # [S, D_OUT]
where A = lora_A reshaped [E*R, D_IN], Bt[(e,r), o] = lora_B[e, o, r],
g_rep[(e,r)] = g_b[e].  The merged per-sample weights of the reference
(W + sum_e g[b,e] * lora_B[e] @ lora_A[e]) are never materialized.

All matmul operands are bf16 (same 1 cycle/row PE rate as float32r but
half the HBM traffic + FWL-accelerated LDWEIGHTS: measured 216 ns per
512-col matmul = the streaming roofline, vs 234 ns for float32r whose
186 ns LDWEIGHTS can't fully hide).  PSUM accumulates in f32; the
output is stored bf16 and widened on the host (rel err ~2.7e-3, budget
2e-2).  x and W are host-packed so every DMA descriptor is an 8KB/4KB
contiguous partition row (descriptor count, not bytes, limits early
DMA).  Schedule: a block of dummy warm-up matmuls keeps the PE HAM
activity window busy while x streams in (else the first ~3.4us of real
matmuls run at 1.2 GHz); o_tiles 0-3 run base-only in K-split phases
sized to the early x arrival rate; the gate's column sums are emitted
in [128,512] pieces placed (in scheduler priority order) between phase
matmuls and the PSUM->SBUF copies the PE waits on; the deferred-lora
vector adds interleave with the first steady tiles; the final stores
drain through the by-then-idle scalar HWDGE ring (the gpsimd SWDGE
path pays a ~2.5us Q7 drain on the critical path).
"""

import sys

import numpy as np

try:
    import concourse.bass  # noqa: F401
except ImportError:  # pragma: no cover - fallback for bare environments
    for _p in (
        "/root/.axon_site",
        "/root/.axon_site/_ro/trn_rl_repo",
        "/root/.axon_site/_ro/pypackages",
        "/opt/trn_rl_repo",
    ):
        if _p not in sys.path:
            sys.path.append(_p)

import ml_dtypes
import concourse.bass as bass  # noqa: F401
import concourse.mybir as mybir
import concourse.tile as tile
from concourse import bacc, bass_utils

S, B, D_IN, D_OUT, E, R = 2048, 8, 1024, 4096, 8, 16
NCORES = 8
ER = E * R            # 128 (one partition dim worth of lora rows)
KC = D_IN // 128      # 8 contraction chunks
NQ = KC // 2          # 4 chunk pairs (x DMA granularity)
NOT = D_OUT // 128    # 32 output tiles
NG = NOT // 2         # 16 weight-tile pairs (wt DMA granularity)
SC = 512              # s-chunk (one PSUM bank of f32)
NSC = S // SC         # 4

F32 = mybir.dt.float32
BF16 = mybir.dt.bfloat16

Ident = mybir.ActivationFunctionType.Identity
CopyF = mybir.ActivationFunctionType.Copy

N_WARM = 100         # dummy matmuls warming the PE HAM window during x DMA


def _build_nc(n_cores: int = NCORES):
    nc = bacc.Bacc(
        "TRN2", target_bir_lowering=False, debug=False, num_devices=n_cores
    )

    xP = nc.dram_tensor("xP", [NQ, 128, 2, S], BF16, kind="ExternalInput").ap()
    WP = nc.dram_tensor(
        "WP", [NG, 128, 2, KC, 128], BF16, kind="ExternalInput"
    ).ap()
    AT = nc.dram_tensor("AT", [128, KC, ER], BF16, kind="ExternalInput").ap()
    Bt = nc.dram_tensor("Bt", [ER, D_OUT], BF16, kind="ExternalInput").ap()
    gwT = nc.dram_tensor("gwT", [128, KC, ER], F32, kind="ExternalInput").ap()
    bias_gb = nc.dram_tensor("bias_gb", [128, NOT + 1], F32, kind="ExternalInput").ap()
    outT = nc.dram_tensor("outT", [D_OUT, S], BF16, kind="ExternalOutput").ap()

    NDEFER = 4  # leading o_tiles processed base-only; lora added later

    with (
        tile.TileContext(nc) as tc,
        tc.tile_pool(name="singles", bufs=1) as singles,
        tc.tile_pool(name="wpool", bufs=3) as wpool,
        tc.tile_pool(name="opool", bufs=3) as opool,
        tc.tile_pool(name="odefer", bufs=NDEFER) as odefer,
        tc.tile_pool(name="ps_a", bufs=4, space="PSUM") as ps_a,
        tc.tile_pool(name="ps_b", bufs=4, space="PSUM") as ps_b,
    ):
        # ---- PE warm-up: dummy matmuls on a zeroed tile keep the HAM
        # activity window busy while x streams in, so the first real
        # matmul runs at 2.4 GHz instead of 1.2 GHz.
        warm = singles.tile([128, 128], BF16)
        nc.vector.memset(warm[:], 0)
        warm_ps_a = ps_a.tile([128, 128], F32, tag="acc")
        warm_ps_b = ps_b.tile([128, 128], F32, tag="acc")
        warm_ps = [warm_ps_a, warm_ps_b]
        for i in range(N_WARM):
            # the 2-bank rotation makes each warm matmul wait out the PSUM
            # write-after-write latency (~173ns cadence): cheap instructions
            # that cover a long wall-clock window
            nc.tensor.matmul(
                warm_ps[i % 2][:], warm[:], warm[:], start=True, stop=True
            )

        # ---- x^T resident in SBUF, loaded as chunk pairs whose partition
        # rows are 8KB contiguous in DRAM (host-packed)
        x_sb = singles.tile([128, KC, S], BF16)
        xsum4 = singles.tile([128, KC, NSC], F32)
        scratch = singles.tile([128, SC], BF16)
        # per-chunk triggers: arrival granularity matches phase 0's
        # consumption rate (~8 matmuls per chunk), so the PE never waits
        # long enough for the HAM window to re-throttle
        for c in range(KC):
            nc.sync.dma_start(
                out=x_sb[:, c : c + 1, :], in_=xP[c // 2][:, c % 2 : c % 2 + 1, :]
            )

        # ---- weight-tile pairs, 4KB contiguous partition rows
        _wt_cache = {}

        def wtp_load(g):
            if g in _wt_cache:
                return _wt_cache.pop(g)
            wtp = wpool.tile([128, 2, KC, 128], BF16, tag="wt")
            nc.scalar.dma_start(out=wtp[:], in_=WP[g])
            return wtp

        for _g in (0, 1):
            _wtp = wpool.tile([128, 2, KC, 128], BF16, tag="wt")
            nc.scalar.dma_start(out=_wtp[:], in_=WP[_g])
            _wt_cache[_g] = _wtp

        # small/late tensors ride the scalar ring behind the first weight
        # pair; the sync ring stays pure-x so the first chunks land as
        # early as possible
        bias_sb = singles.tile([128, NOT + 1], F32)
        nc.scalar.dma_start(out=bias_sb[:], in_=bias_gb)
        gb_sb = bias_sb[:, NOT : NOT + 1]
        # at/gw/Bt are needed only ~40-48us in: they ride the sync ring
        # BEHIND x so they never compete with the startup-critical chunks
        at_sb = singles.tile([128, KC, ER], BF16)
        nc.sync.dma_start(out=at_sb[:], in_=AT[:])
        gw_sb = singles.tile([128, KC, ER], F32)
        nc.sync.dma_start(out=gw_sb[:], in_=gwT[:])
        bt_sb = singles.tile([128, D_OUT], BF16)
        nc.sync.dma_start(out=bt_sb[:], in_=Bt)

        # xsum4[:, c, sc] = per-(chunk, s-quarter) column sums for the gate.
        # Issued in small [128,512] pieces AFTER the defer phases below (see
        # emit_xsum) so the scheduler never lets them preempt the PSUM->SBUF
        # copies the PE is waiting on; the 4 partials per chunk ride through
        # the gate matmul's free dim and are reduced at the end.
        def emit_xsum(c_lo, c_hi):
            for c in range(c_lo, c_hi):
                for sc in range(NSC):
                    sl = slice(sc * SC, (sc + 1) * SC)
                    if (c + sc) % 2 == 0:
                        nc.vector.reduce_sum(
                            out=xsum4[:, c, sc : sc + 1],
                            in_=x_sb[:, c, sl],
                            axis=mybir.AxisListType.X,
                        )
                    else:
                        nc.scalar.activation(
                            out=scratch[:, :SC],
                            in_=x_sb[:, c, sl],
                            func=CopyF,
                            accum_out=xsum4[:, c, sc : sc + 1],
                        )

        def base_mms(accs, wt, stop):
            for c in range(KC):
                for sc in range(NSC):
                    nc.tensor.matmul(
                        accs[sc][:],
                        wt[:, c, :],
                        x_sb[:, c, sc * SC : (sc + 1) * SC],
                        start=(c == 0),
                        stop=(stop and c == KC - 1),
                    )

        def bias_copy(o_sb, accs, ot):
            for sc in range(NSC):
                sl = slice(sc * SC, (sc + 1) * SC)
                if (ot + sc) % 2 == 0:
                    nc.vector.tensor_scalar_add(
                        o_sb[:, sl], accs[sc][:], bias_sb[:, ot : ot + 1]
                    )
                else:
                    nc.scalar.activation(
                        out=o_sb[:, sl],
                        in_=accs[sc][:],
                        func=Ident,
                        bias=bias_sb[:, ot : ot + 1],
                        scale=1.0,
                    )

        _ps_toggle = [0]

        def psum_group():
            pool = ps_a if _ps_toggle[0] % 2 == 0 else ps_b
            _ps_toggle[0] += 1
            accs = []
            for _sc in range(NSC):
                acc = pool.tile([128, SC], F32, tag="acc")
                accs.append(acc)
            return accs

        # ---- deferred o_tiles ot0/ot1: base-only matmuls, K-split into
        # chunk halves (c 0-3 then c 4-7) so PSUM banks recycle mid-load
        # and the PE always has dense work while x streams in.  A-half
        # carries the bias; B-half is added.  Their lora term lands later,
        # once the gate is known.
        defer_wtp = [wtp_load(0), wtp_load(1)]
        defer_o = []
        for ot in range(NDEFER):
            o_sb = odefer.tile([128, S], BF16, tag="od")
            defer_o.append(o_sb)
        KH = KC // 2
        for phase, (ots, c_lo, c_hi) in enumerate(
            [
                ((0, 1), 0, KH),
                ((2, 3), 0, KH),
                ((0, 1), KH, KC),
                ((2, 3), KH, KC),
            ]
        ):
            groups = {}
            for ot in ots:
                groups[ot] = psum_group()
            # chunk-major across the ot pair: 8 ready matmuls per arriving
            # x chunk instead of 4 (the PE stream is in-order)
            for c in range(c_lo, c_hi):
                for ot in ots:
                    for sc in range(NSC):
                        nc.tensor.matmul(
                            groups[ot][sc][:],
                            defer_wtp[ot // 2][:, ot % 2, c, :],
                            x_sb[:, c, sc * SC : (sc + 1) * SC],
                            start=(c == c_lo),
                            stop=(c == c_hi - 1),
                        )
            if phase == 1:
                # c0-3 column sums sit between phase 1's matmuls and its
                # copies in priority order: the Vector/Scalar engines run
                # them during phase 1's matmul span, after phase 0's
                # copies but never ahead of them
                emit_xsum(0, KH)
            elif phase == 3:
                emit_xsum(KH, KC)
            for ot in ots:
                if c_lo == 0:
                    bias_copy(defer_o[ot], groups[ot], ot)
                else:
                    for sc in range(NSC):
                        sl = slice(sc * SC, (sc + 1) * SC)
                        nc.vector.tensor_add(
                            defer_o[ot][:, sl], defer_o[ot][:, sl], groups[ot][sc][:]
                        )

        # ---- u^T[er, s] = A @ x_b^T  (needs all of x, only PE + copies).
        # sc-outer: each bank takes its full 8-chunk accumulation run as
        # soon as phase 1's adds free it, instead of all four banks gating
        # the first four matmuls.
        u_sb = singles.tile([128, S], BF16)
        uaccs = psum_group()
        for sc in range(NSC):
            for c in range(KC):
                nc.tensor.matmul(
                    uaccs[sc][:],
                    at_sb[:, c, :],
                    x_sb[:, c, sc * SC : (sc + 1) * SC],
                    start=(c == 0),
                    stop=(c == KC - 1),
                )
            # scalar engine: the vector engine is loaded with lora adds here
            nc.scalar.activation(
                out=u_sb[:, sc * SC : (sc + 1) * SC],
                in_=uaccs[sc][:],
                func=Ident,
                scale=1.0,
            )

        # ---- gate: g4[er, sc] = sum_c gw_sb[:,c,:]^T @ xsum4[:,c,sc], then
        # g = sum_sc g4 + gb (gwT is pre-scaled by 1/S on the host)
        g_ps = ps_b.tile([128, NSC], F32, tag="acc")
        for c in range(KC):
            nc.tensor.matmul(
                g_ps[:],
                gw_sb[:, c, :],
                xsum4[:, c, :],
                start=(c == 0),
                stop=(c == KC - 1),
            )
        g_r = singles.tile([128, 1], F32)
        nc.vector.reduce_sum(out=g_r[:], in_=g_ps[:], axis=mybir.AxisListType.X)
        g_sb = singles.tile([128, 1], F32)
        nc.vector.tensor_add(g_sb[:], g_r[:], gb_sb)

        # fold the gate into Bt: bts[er, o] = g[er] * Bt[er, o]  (bf16),
        # split across Vector and Scalar engines; runs while the PE does
        # the u matmuls below
        bts_sb = singles.tile([128, D_OUT], BF16)
        half = D_OUT // 2
        nc.vector.tensor_scalar_mul(bts_sb[:, :half], bt_sb[:, :half], g_sb[:])
        nc.scalar.activation(
            out=bts_sb[:, half:],
            in_=bt_sb[:, half:],
            func=Ident,
            scale=g_sb[:],
        )

        # ---- lora for one deferred o_tile, then store it
        def defer_lora(ot):
            osl = slice(ot * 128, (ot + 1) * 128)
            laccs = psum_group()
            for sc in range(NSC):
                nc.tensor.matmul(
                    laccs[sc][:],
                    bts_sb[:, osl],
                    u_sb[:, sc * SC : (sc + 1) * SC],
                    start=True,
                    stop=True,
                )
            for sc in range(NSC):
                sl = slice(sc * SC, (sc + 1) * SC)
                nc.vector.tensor_add(
                    defer_o[ot][:, sl], defer_o[ot][:, sl], laccs[sc][:]
                )
            nc.sync.dma_start(out=outT[osl, :], in_=defer_o[ot][:])

        # ---- one steady-state fused tile
        def steady(ot):
            wtp = wtp_load(ot // 2)
            if ot % 2 == 0:
                _wt_cache[ot // 2] = wtp
            o_sb = opool.tile([128, S], BF16, tag="o")
            osl = slice(ot * 128, (ot + 1) * 128)
            accs = psum_group()
            for c in range(KC):
                for sc in range(NSC):
                    nc.tensor.matmul(
                        accs[sc][:],
                        wtp[:, ot % 2, c, :],
                        x_sb[:, c, sc * SC : (sc + 1) * SC],
                        start=(c == 0),
                        stop=False,
                    )
            for sc in range(NSC):
                nc.tensor.matmul(
                    accs[sc][:],
                    bts_sb[:, osl],
                    u_sb[:, sc * SC : (sc + 1) * SC],
                    start=False,
                    stop=True,
                )
            bias_copy(o_sb, accs, ot)
            if ot == NOT - 1:
                # last tile: stored in s-halves via the scalar HWDGE ring
                # (idle and drained by now; the gpsimd SWDGE path pays a
                # ~2.5us Q7 drain that would land on the critical path)
                nc.scalar.dma_start(out=outT[osl, : S // 2], in_=o_sb[:, : S // 2])
                nc.scalar.dma_start(out=outT[osl, S // 2 :], in_=o_sb[:, S // 2 :])
            elif ot >= NOT - 3:
                # penultimate tiles drain through the scalar ring, idle
                # once the weight loads finish
                nc.scalar.dma_start(out=outT[osl, :], in_=o_sb[:])
            else:
                nc.sync.dma_start(out=outT[osl, :], in_=o_sb[:])

        # interleave the deferred-lora groups (vector-add heavy) with the
        # first steady tiles (matmul heavy) so the Vector engine never
        # becomes the PE's bottleneck in the transition
        defer_lora(0)
        defer_lora(1)
        steady(4)
        defer_lora(2)
        steady(5)
        defer_lora(3)
        for ot in range(6, NOT):
            steady(ot)

    nc.compile()
    return nc


def _prep_in_maps(x, gate_w, gate_b, W, bias, lora_A, lora_B):
    f32 = np.float32
    bf16 = ml_dtypes.bfloat16
    x = np.asarray(x, f32)
    gate_w = np.asarray(gate_w, f32)
    gate_b = np.asarray(gate_b, f32)
    W = np.asarray(W, f32)
    bias = np.asarray(bias, f32)
    lora_A = np.asarray(lora_A, f32)
    lora_B = np.asarray(lora_B, f32)

    # WP[g, p, i, c, m] = W[(2g+i)*128 + m, c*128 + p]: per (g, p) the
    # (i, c, m) block is 4KB contiguous -> big DMA descriptors
    WP = np.ascontiguousarray(
        W.reshape(NG, 2, 128, KC, 128).transpose(0, 4, 1, 3, 2)
    ).astype(bf16)
    AT = np.ascontiguousarray(
        lora_A.reshape(ER, D_IN).T.reshape(KC, 128, ER).transpose(1, 0, 2)
    ).astype(bf16)
    Bt = np.ascontiguousarray(lora_B.transpose(0, 2, 1).reshape(ER, D_OUT)).astype(
        bf16
    )
    gwT = np.ascontiguousarray(
        (np.repeat(gate_w, R, axis=0).T / np.float32(S))
        .reshape(KC, 128, ER)
        .transpose(1, 0, 2)
    )
    gbr = np.repeat(gate_b, R).reshape(ER, 1)
    bias_t = bias.reshape(NOT, 128).T
    bias_gb = np.ascontiguousarray(
        np.concatenate([bias_t, gbr], axis=1).astype(f32)
    )

    shared = {
        "WP": WP,
        "AT": AT,
        "Bt": Bt,
        "gwT": gwT.astype(f32),
        "bias_gb": bias_gb,
    }
    in_maps = []
    for b in range(NCORES):
        m = dict(shared)
        # xP[q, p, cc, s] = x[s, b, (2q+cc)*128 + p]: per (q, p) the
        # (cc, s) block is 8KB contiguous
        xb = np.ascontiguousarray(x[:, b, :].T).astype(bf16)
        m["xP"] = np.ascontiguousarray(
            xb.reshape(NQ, 2, 128, S).transpose(0, 2, 1, 3)
        )
        in_maps.append(m)
    return in_maps


def run(inputs, trace=False, trace_cores=None):
    """Build + run on 8 cores. Returns (out [S,B,D_OUT], BassKernelResults)."""
    in_maps = _prep_in_maps(**inputs)
    nc = _build_nc()
    kwargs = {}
    if trace:
        _register_axon_ntff_hook()
        kwargs = dict(trace=True, trace_cores=trace_cores or [0])
    res = bass_utils.run_bass_kernel_spmd(
        nc, in_maps, core_ids=list(range(NCORES)), **kwargs
    )
    out = np.empty((S, B, D_OUT), np.float32)
    for b in range(NCORES):
        out[:, b, :] = res.results[b]["outT"].T.astype(np.float32)
    return out, res


def _register_axon_ntff_hook():
    """antenv.axon_hooks is missing on this image; synthesize it so
    run_bass_kernel_spmd(trace=True) can reach the axon NTFF profiler."""
    import types

    try:
        from antenv.axon_hooks import get_axon_ntff_profile_hook  # noqa: F401

        return  # real module present
    except ImportError:
        pass
    try:
        from trn_agent_boot.trn_boot import _ntff_profile_via_ctypes
    except ImportError:
        return
    import antenv

    mod = types.ModuleType("antenv.axon_hooks")
    _state = {"hook": None}
    mod.set_axon_ntff_profile_hook = lambda h: _state.__setitem__("hook", h)
    mod.get_axon_ntff_profile_hook = lambda: _state["hook"]
    sys.modules["antenv.axon_hooks"] = mod
    antenv.axon_hooks = mod
    hook = _ntff_profile_via_ctypes("/opt/axon/libaxon_pjrt.so")
    if hook is not None:
        mod.set_axon_ntff_profile_hook(hook)


def kernel(**inputs) -> np.ndarray:
    out, _ = run(inputs, trace=False)
    return out


# revision 49
# speedup vs baseline: 1.0044x; 1.0044x over previous
"""LoRA-MoE fused kernel for 8x Trainium2 NeuronCores (Bass/Tile).

Math (per batch sample b, data-parallel across 8 cores):
    g_b    = gate_w @ mean_s(x_b) + gate_b                      # [E]
    out_b  = x_b @ W^T + ((x_b @ A^T) * g_rep) @ Bt + bias      # [S, D_OUT]
where A = lora_A reshaped [E*R, D_IN], Bt[(e,r), o] = lora_B[e, o, r],
g_rep[(e,r)] = g_b[e].  The merged per-sample weights of the reference
(W + sum_e g[b,e] * lora_B[e] @ lora_A[e]) are never materialized.

All matmul operands are bf16 (same 1 cycle/row PE rate as float32r but
half the HBM traffic + FWL-accelerated LDWEIGHTS: measured 216 ns per
512-col matmul = the streaming roofline, vs 234 ns for float32r whose
186 ns LDWEIGHTS can't fully hide).  PSUM accumulates in f32; the
output is stored bf16 and widened on the host (rel err ~2.7e-3, budget
2e-2).  x and W are host-packed so every DMA descriptor is an 8KB/4KB
contiguous partition row (descriptor count, not bytes, limits early
DMA).  Schedule: a block of dummy warm-up matmuls keeps the PE HAM
activity window busy while x streams in (else the first ~3.4us of real
matmuls run at 1.2 GHz); o_tiles 0-3 run base-only in K-split phases
sized to the early x arrival rate; the gate's column sums are emitted
in [128,512] pieces placed (in scheduler priority order) between phase
matmuls and the PSUM->SBUF copies the PE waits on; the deferred-lora
vector adds interleave with the first steady tiles; the final stores
drain through the by-then-idle scalar HWDGE ring (the gpsimd SWDGE
path pays a ~2.5us Q7 drain on the critical path).
"""

import sys

import numpy as np

try:
    import concourse.bass  # noqa: F401
except ImportError:  # pragma: no cover - fallback for bare environments
    for _p in (
        "/root/.axon_site",
        "/root/.axon_site/_ro/trn_rl_repo",
        "/root/.axon_site/_ro/pypackages",
        "/opt/trn_rl_repo",
    ):
        if _p not in sys.path:
            sys.path.append(_p)

import ml_dtypes
import concourse.bass as bass  # noqa: F401
import concourse.mybir as mybir
import concourse.tile as tile
from concourse import bacc, bass_utils

S, B, D_IN, D_OUT, E, R = 2048, 8, 1024, 4096, 8, 16
NCORES = 8
ER = E * R            # 128 (one partition dim worth of lora rows)
KC = D_IN // 128      # 8 contraction chunks
NQ = KC // 2          # 4 chunk pairs (x DMA granularity)
NOT = D_OUT // 128    # 32 output tiles
NG = NOT // 2         # 16 weight-tile pairs (wt DMA granularity)
SC = 512              # s-chunk (one PSUM bank of f32)
NSC = S // SC         # 4

F32 = mybir.dt.float32
BF16 = mybir.dt.bfloat16

Ident = mybir.ActivationFunctionType.Identity
CopyF = mybir.ActivationFunctionType.Copy

N_WARM = 100         # dummy matmuls warming the PE HAM window during x DMA


def _build_nc(n_cores: int = NCORES):
    nc = bacc.Bacc(
        "TRN2", target_bir_lowering=False, debug=False, num_devices=n_cores
    )

    xP = nc.dram_tensor("xP", [NQ, 128, 2, S], BF16, kind="ExternalInput").ap()
    WP = nc.dram_tensor(
        "WP", [NG, 128, 2, KC, 128], BF16, kind="ExternalInput"
    ).ap()
    AT = nc.dram_tensor("AT", [128, KC, ER], BF16, kind="ExternalInput").ap()
    Bt = nc.dram_tensor("Bt", [ER, D_OUT], BF16, kind="ExternalInput").ap()
    gwT = nc.dram_tensor("gwT", [128, KC, ER], F32, kind="ExternalInput").ap()
    # weight pair 0 with bias+gate_b packed as trailing bf16 columns: one
    # 4KB-row transfer instead of wtp0 plus 128 tiny bias descriptors
    W0X = nc.dram_tensor(
        "W0X", [128, 2 * KC * 128 + NOT + 1], BF16, kind="ExternalInput"
    ).ap()
    outT = nc.dram_tensor("outT", [D_OUT, S], BF16, kind="ExternalOutput").ap()

    NDEFER = 4  # leading o_tiles processed base-only; lora added later

    with (
        tile.TileContext(nc) as tc,
        tc.tile_pool(name="singles", bufs=1) as singles,
        tc.tile_pool(name="wpool", bufs=3) as wpool,
        tc.tile_pool(name="opool", bufs=3) as opool,
        tc.tile_pool(name="odefer", bufs=NDEFER) as odefer,
        tc.tile_pool(name="ps_a", bufs=4, space="PSUM") as ps_a,
        tc.tile_pool(name="ps_b", bufs=4, space="PSUM") as ps_b,
    ):
        # ---- PE warm-up: dummy matmuls on a zeroed tile keep the HAM
        # activity window busy while x streams in, so the first real
        # matmul runs at 2.4 GHz instead of 1.2 GHz.
        warm = singles.tile([128, 128], BF16)
        nc.vector.memset(warm[:], 0)
        warm_ps_a = ps_a.tile([128, 128], F32, tag="acc")
        warm_ps_b = ps_b.tile([128, 128], F32, tag="acc")
        warm_ps = [warm_ps_a, warm_ps_b]
        for i in range(N_WARM):
            # the 2-bank rotation makes each warm matmul wait out the PSUM
            # write-after-write latency (~173ns cadence): cheap instructions
            # that cover a long wall-clock window
            nc.tensor.matmul(
                warm_ps[i % 2][:], warm[:], warm[:], start=True, stop=True
            )

        # ---- x^T resident in SBUF, loaded as chunk pairs whose partition
        # rows are 8KB contiguous in DRAM (host-packed)
        x_sb = singles.tile([128, KC, S], BF16)
        xsum4 = singles.tile([128, KC, NSC], F32)
        scratch = singles.tile([128, SC], BF16)
        # per-chunk triggers: arrival granularity matches phase 0's
        # consumption rate (~8 matmuls per chunk), so the PE never waits
        # long enough for the HAM window to re-throttle
        for c in range(KC):
            nc.sync.dma_start(
                out=x_sb[:, c : c + 1, :], in_=xP[c // 2][:, c % 2 : c % 2 + 1, :]
            )

        # ---- weight-tile pairs, 4KB contiguous partition rows
        _wt_cache = {}

        def wtp_load(g):
            if g in _wt_cache:
                return _wt_cache.pop(g)
            wtp = wpool.tile([128, 2, KC, 128], BF16, tag="wt")
            nc.scalar.dma_start(out=wtp[:], in_=WP[g])
            return wtp

        w0x_sb = singles.tile([128, 2 * KC * 128 + NOT + 1], BF16)
        nc.scalar.dma_start(out=w0x_sb[:], in_=W0X)
        _wtp1 = wpool.tile([128, 2, KC, 128], BF16, tag="wt")
        nc.scalar.dma_start(out=_wtp1[:], in_=WP[1])
        _wt_cache[1] = _wtp1

        # widen the packed bias+gate_b to f32 once w0x lands (one cheap
        # vector op; everything downstream keeps its f32 view)
        bias_sb = singles.tile([128, NOT + 1], F32)
        nc.vector.tensor_copy(bias_sb[:], w0x_sb[:, 2 * KC * 128 :])
        gb_sb = bias_sb[:, NOT : NOT + 1]
        # at/gw/Bt are needed only ~40-48us in: they ride the sync ring
        # BEHIND x so they never compete with the startup-critical chunks
        at_sb = singles.tile([128, KC, ER], BF16)
        nc.sync.dma_start(out=at_sb[:], in_=AT[:])
        gw_sb = singles.tile([128, KC, ER], F32)
        nc.sync.dma_start(out=gw_sb[:], in_=gwT[:])
        bt_sb = singles.tile([128, D_OUT], BF16)
        nc.sync.dma_start(out=bt_sb[:], in_=Bt)

        # xsum4[:, c, sc] = per-(chunk, s-quarter) column sums for the gate.
        # Issued in small [128,512] pieces AFTER the defer phases below (see
        # emit_xsum) so the scheduler never lets them preempt the PSUM->SBUF
        # copies the PE is waiting on; the 4 partials per chunk ride through
        # the gate matmul's free dim and are reduced at the end.
        def emit_xsum(c_lo, c_hi):
            for c in range(c_lo, c_hi):
                for sc in range(NSC):
                    sl = slice(sc * SC, (sc + 1) * SC)
                    if (c + sc) % 2 == 0:
                        nc.vector.reduce_sum(
                            out=xsum4[:, c, sc : sc + 1],
                            in_=x_sb[:, c, sl],
                            axis=mybir.AxisListType.X,
                        )
                    else:
                        nc.scalar.activation(
                            out=scratch[:, :SC],
                            in_=x_sb[:, c, sl],
                            func=CopyF,
                            accum_out=xsum4[:, c, sc : sc + 1],
                        )

        def base_mms(accs, wt, stop):
            for c in range(KC):
                for sc in range(NSC):
                    nc.tensor.matmul(
                        accs[sc][:],
                        wt[:, c, :],
                        x_sb[:, c, sc * SC : (sc + 1) * SC],
                        start=(c == 0),
                        stop=(stop and c == KC - 1),
                    )

        def bias_copy(o_sb, accs, ot):
            for sc in range(NSC):
                sl = slice(sc * SC, (sc + 1) * SC)
                if (ot + sc) % 2 == 0:
                    nc.vector.tensor_scalar_add(
                        o_sb[:, sl], accs[sc][:], bias_sb[:, ot : ot + 1]
                    )
                else:
                    nc.scalar.activation(
                        out=o_sb[:, sl],
                        in_=accs[sc][:],
                        func=Ident,
                        bias=bias_sb[:, ot : ot + 1],
                        scale=1.0,
                    )

        _ps_toggle = [0]

        def psum_group():
            pool = ps_a if _ps_toggle[0] % 2 == 0 else ps_b
            _ps_toggle[0] += 1
            accs = []
            for _sc in range(NSC):
                acc = pool.tile([128, SC], F32, tag="acc")
                accs.append(acc)
            return accs

        # ---- deferred o_tiles ot0/ot1: base-only matmuls, K-split into
        # chunk halves (c 0-3 then c 4-7) so PSUM banks recycle mid-load
        # and the PE always has dense work while x streams in.  A-half
        # carries the bias; B-half is added.  Their lora term lands later,
        # once the gate is known.
        defer_wtp1 = wtp_load(1)

        def defer_wt_ap(ot, c):
            if ot < 2:
                base = ot * (KC * 128) + c * 128
                return w0x_sb[:, base : base + 128]
            return defer_wtp1[:, ot % 2, c, :]

        defer_o = []
        for ot in range(NDEFER):
            o_sb = odefer.tile([128, S], BF16, tag="od")
            defer_o.append(o_sb)
        KH = KC // 2
        for phase, (ots, c_lo, c_hi) in enumerate(
            [
                ((0, 1), 0, KH),
                ((2, 3), 0, KH),
                ((0, 1), KH, KC),
                ((2, 3), KH, KC),
            ]
        ):
            groups = {}
            for ot in ots:
                groups[ot] = psum_group()
            # chunk-major across the ot pair: 8 ready matmuls per arriving
            # x chunk instead of 4 (the PE stream is in-order)
            for c in range(c_lo, c_hi):
                for ot in ots:
                    for sc in range(NSC):
                        nc.tensor.matmul(
                            groups[ot][sc][:],
                            defer_wt_ap(ot, c),
                            x_sb[:, c, sc * SC : (sc + 1) * SC],
                            start=(c == c_lo),
                            stop=(c == c_hi - 1),
                        )
            if phase == 1:
                # c0-3 column sums sit between phase 1's matmuls and its
                # copies in priority order: the Vector/Scalar engines run
                # them during phase 1's matmul span, after phase 0's
                # copies but never ahead of them
                emit_xsum(0, KH)
            elif phase == 3:
                emit_xsum(KH, KC)
            for ot in ots:
                if c_lo == 0:
                    bias_copy(defer_o[ot], groups[ot], ot)
                else:
                    for sc in range(NSC):
                        sl = slice(sc * SC, (sc + 1) * SC)
                        nc.vector.tensor_add(
                            defer_o[ot][:, sl], defer_o[ot][:, sl], groups[ot][sc][:]
                        )

        # ---- u^T[er, s] = A @ x_b^T  (needs all of x, only PE + copies).
        # sc-outer: each bank takes its full 8-chunk accumulation run as
        # soon as phase 1's adds free it, instead of all four banks gating
        # the first four matmuls.
        u_sb = singles.tile([128, S], BF16)
        uaccs = psum_group()
        for sc in range(NSC):
            for c in range(KC):
                nc.tensor.matmul(
                    uaccs[sc][:],
                    at_sb[:, c, :],
                    x_sb[:, c, sc * SC : (sc + 1) * SC],
                    start=(c == 0),
                    stop=(c == KC - 1),
                )
            # scalar engine: the vector engine is loaded with lora adds here
            nc.scalar.activation(
                out=u_sb[:, sc * SC : (sc + 1) * SC],
                in_=uaccs[sc][:],
                func=Ident,
                scale=1.0,
            )

        # ---- gate: g4[er, sc] = sum_c gw_sb[:,c,:]^T @ xsum4[:,c,sc], then
        # g = sum_sc g4 + gb (gwT is pre-scaled by 1/S on the host)
        g_ps = ps_b.tile([128, NSC], F32, tag="acc")
        for c in range(KC):
            nc.tensor.matmul(
                g_ps[:],
                gw_sb[:, c, :],
                xsum4[:, c, :],
                start=(c == 0),
                stop=(c == KC - 1),
            )
        g_r = singles.tile([128, 1], F32)
        nc.vector.reduce_sum(out=g_r[:], in_=g_ps[:], axis=mybir.AxisListType.X)
        g_sb = singles.tile([128, 1], F32)
        nc.vector.tensor_add(g_sb[:], g_r[:], gb_sb)

        # fold the gate into Bt: bts[er, o] = g[er] * Bt[er, o]  (bf16),
        # split across Vector and Scalar engines; runs while the PE does
        # the u matmuls below
        bts_sb = singles.tile([128, D_OUT], BF16)
        half = D_OUT // 2
        nc.vector.tensor_scalar_mul(bts_sb[:, :half], bt_sb[:, :half], g_sb[:])
        nc.scalar.activation(
            out=bts_sb[:, half:],
            in_=bt_sb[:, half:],
            func=Ident,
            scale=g_sb[:],
        )

        # ---- lora for one deferred o_tile, then store it
        def defer_lora(ot):
            osl = slice(ot * 128, (ot + 1) * 128)
            laccs = psum_group()
            for sc in range(NSC):
                nc.tensor.matmul(
                    laccs[sc][:],
                    bts_sb[:, osl],
                    u_sb[:, sc * SC : (sc + 1) * SC],
                    start=True,
                    stop=True,
                )
            for sc in range(NSC):
                sl = slice(sc * SC, (sc + 1) * SC)
                nc.vector.tensor_add(
                    defer_o[ot][:, sl], defer_o[ot][:, sl], laccs[sc][:]
                )
            nc.sync.dma_start(out=outT[osl, :], in_=defer_o[ot][:])

        # ---- one steady-state fused tile
        def steady(ot):
            wtp = wtp_load(ot // 2)
            if ot % 2 == 0:
                _wt_cache[ot // 2] = wtp
            o_sb = opool.tile([128, S], BF16, tag="o")
            osl = slice(ot * 128, (ot + 1) * 128)
            accs = psum_group()
            for c in range(KC):
                for sc in range(NSC):
                    nc.tensor.matmul(
                        accs[sc][:],
                        wtp[:, ot % 2, c, :],
                        x_sb[:, c, sc * SC : (sc + 1) * SC],
                        start=(c == 0),
                        stop=False,
                    )
            for sc in range(NSC):
                nc.tensor.matmul(
                    accs[sc][:],
                    bts_sb[:, osl],
                    u_sb[:, sc * SC : (sc + 1) * SC],
                    start=False,
                    stop=True,
                )
            bias_copy(o_sb, accs, ot)
            if ot == NOT - 1:
                # last tile: stored in s-halves via the scalar HWDGE ring
                # (idle and drained by now; the gpsimd SWDGE path pays a
                # ~2.5us Q7 drain that would land on the critical path)
                nc.scalar.dma_start(out=outT[osl, : S // 2], in_=o_sb[:, : S // 2])
                nc.scalar.dma_start(out=outT[osl, S // 2 :], in_=o_sb[:, S // 2 :])
            elif ot >= NOT - 3:
                # penultimate tiles drain through the scalar ring, idle
                # once the weight loads finish
                nc.scalar.dma_start(out=outT[osl, :], in_=o_sb[:])
            else:
                nc.sync.dma_start(out=outT[osl, :], in_=o_sb[:])

        # interleave the deferred-lora groups (vector-add heavy) with the
        # first steady tiles (matmul heavy) so the Vector engine never
        # becomes the PE's bottleneck in the transition
        defer_lora(0)
        defer_lora(1)
        steady(4)
        defer_lora(2)
        steady(5)
        defer_lora(3)
        for ot in range(6, NOT):
            steady(ot)

    nc.compile()
    return nc


def _prep_in_maps(x, gate_w, gate_b, W, bias, lora_A, lora_B):
    f32 = np.float32
    bf16 = ml_dtypes.bfloat16
    x = np.asarray(x, f32)
    gate_w = np.asarray(gate_w, f32)
    gate_b = np.asarray(gate_b, f32)
    W = np.asarray(W, f32)
    bias = np.asarray(bias, f32)
    lora_A = np.asarray(lora_A, f32)
    lora_B = np.asarray(lora_B, f32)

    # WP[g, p, i, c, m] = W[(2g+i)*128 + m, c*128 + p]: per (g, p) the
    # (i, c, m) block is 4KB contiguous -> big DMA descriptors
    WP = np.ascontiguousarray(
        W.reshape(NG, 2, 128, KC, 128).transpose(0, 4, 1, 3, 2)
    ).astype(bf16)
    AT = np.ascontiguousarray(
        lora_A.reshape(ER, D_IN).T.reshape(KC, 128, ER).transpose(1, 0, 2)
    ).astype(bf16)
    Bt = np.ascontiguousarray(lora_B.transpose(0, 2, 1).reshape(ER, D_OUT)).astype(
        bf16
    )
    gwT = np.ascontiguousarray(
        (np.repeat(gate_w, R, axis=0).T / np.float32(S))
        .reshape(KC, 128, ER)
        .transpose(1, 0, 2)
    )
    gbr = np.repeat(gate_b, R).reshape(ER, 1)
    bias_t = bias.reshape(NOT, 128).T
    bias_gb33 = np.concatenate([bias_t, gbr], axis=1).astype(bf16)
    W0X = np.ascontiguousarray(
        np.concatenate([WP[0].reshape(128, 2 * KC * 128), bias_gb33], axis=1)
    )

    shared = {
        "WP": WP,
        "W0X": W0X,
        "AT": AT,
        "Bt": Bt,
        "gwT": gwT.astype(f32),
    }
    in_maps = []
    for b in range(NCORES):
        m = dict(shared)
        # xP[q, p, cc, s] = x[s, b, (2q+cc)*128 + p]: per (q, p) the
        # (cc, s) block is 8KB contiguous
        xb = np.ascontiguousarray(x[:, b, :].T).astype(bf16)
        m["xP"] = np.ascontiguousarray(
            xb.reshape(NQ, 2, 128, S).transpose(0, 2, 1, 3)
        )
        in_maps.append(m)
    return in_maps


def run(inputs, trace=False, trace_cores=None):
    """Build + run on 8 cores. Returns (out [S,B,D_OUT], BassKernelResults)."""
    in_maps = _prep_in_maps(**inputs)
    nc = _build_nc()
    kwargs = {}
    if trace:
        _register_axon_ntff_hook()
        kwargs = dict(trace=True, trace_cores=trace_cores or [0])
    res = bass_utils.run_bass_kernel_spmd(
        nc, in_maps, core_ids=list(range(NCORES)), **kwargs
    )
    out = np.empty((S, B, D_OUT), np.float32)
    for b in range(NCORES):
        out[:, b, :] = res.results[b]["outT"].T.astype(np.float32)
    return out, res


def _register_axon_ntff_hook():
    """antenv.axon_hooks is missing on this image; synthesize it so
    run_bass_kernel_spmd(trace=True) can reach the axon NTFF profiler."""
    import types

    try:
        from antenv.axon_hooks import get_axon_ntff_profile_hook  # noqa: F401

        return  # real module present
    except ImportError:
        pass
    try:
        from trn_agent_boot.trn_boot import _ntff_profile_via_ctypes
    except ImportError:
        return
    import antenv

    mod = types.ModuleType("antenv.axon_hooks")
    _state = {"hook": None}
    mod.set_axon_ntff_profile_hook = lambda h: _state.__setitem__("hook", h)
    mod.get_axon_ntff_profile_hook = lambda: _state["hook"]
    sys.modules["antenv.axon_hooks"] = mod
    antenv.axon_hooks = mod
    hook = _ntff_profile_via_ctypes("/opt/axon/libaxon_pjrt.so")
    if hook is not None:
        mod.set_axon_ntff_profile_hook(hook)


def kernel(**inputs) -> np.ndarray:
    out, _ = run(inputs, trace=False)
    return out
